# revision 34
# baseline (speedup 1.0000x reference)
"""LlamaAttention (B=1, S=2048, D=2048, H=16, KV=4) on 8 TRN2 NeuronCores.

Tensor-parallel over heads: core c owns q-heads [2c, 2c+1] and kv-head c//2.
Each core computes partial = attn_out_c @ Wo[:, c-slice].T over the full
sequence; the all-reduce after o_proj happens on the host (sum of partials).

v3 design:
  * QKV projections in fp16 (hs + Wq/Wk/Wv fp16), 512-col PSUM quarters,
    chunked hs DMAs ordered so the first matmul starts a few us in.
  * Attention TRANSPOSED: sT[k,q] = krot_j^T qrot (bf16); causal mask added
    to the diagonal block by a PE accumulate-matmul (identity x trit); exp ->
    E^T tiles; PV via lhsT=vnat (natural [s,hd] V) streaming E^T -> aout[hd,q];
    row sums l[q] via M=1 ones-matmuls into a shared [2,512] PSUM row.
    The j-loop runs BOTH heads interleaved with PV/l lagged 2 slots behind
    sT/exp, so the PE never stalls on the scalar engine's exp.
  * l rows bounce through DRAM (2KB) to become [128, .] columns; 1/l scaling
    is a DVE per-partition tensor_scalar on the [q, hd] transposed aout.
  * aout / V transposes use the DMA XBAR (16-bit SBUF->SBUF transpose), not
    the PE: one descriptor transposes a whole [128, 512] tile into 4 stacked
    [q,128] blocks.
  * o_proj (bf16) interleaved per 512-col chunk; bf16 partials out, host sum.
"""
import math
import numpy as np

S = 2048
D = 2048
HD = 128
H = 16
KV = 4
NCORES = 8
NT = S // 128           # 16 sequence tiles
DTC = D // 128          # 16 feature chunks
NQ = 4                  # 512-col sequence quarters
QC = S // NQ            # 512
QH = H // NCORES        # 2 q-heads per core
ROPE_BASE = 10000.0
SCALE = 1.0 / math.sqrt(HD)
NEG = -1.0e9
LAG = 3                 # slots between sT/exp and PV/l in the j pipeline

_CACHE = {}


def _rope(nc, pool, dst, cols, src_ps, cos_sb, sin_sb, F32, ALU):
    """dst[:, cols] = src*cos + rotate_half(src)*sin  (src: psum [128, 512])."""
    w = cols.stop - cols.start
    tmp = pool.tile([128, w], F32, tag="ropetmp")
    nc.scalar.copy(out=tmp[0:64, :], in_=src_ps[64:128, :])
    nc.scalar.copy(out=tmp[64:128, :], in_=src_ps[0:64, :])
    nc.vector.tensor_tensor(out=dst[:, cols], in0=src_ps, in1=cos_sb[:, cols], op=ALU.mult)
    nc.vector.tensor_tensor(out=tmp, in0=tmp, in1=sin_sb[:, cols], op=ALU.mult)
    nc.vector.tensor_tensor(out=dst[:, cols], in0=dst[:, cols], in1=tmp, op=ALU.add)


def build_nc():
    import concourse.bacc as bacc
    import concourse.tile as tile
    from concourse import mybir

    F32 = mybir.dt.float32
    F16 = mybir.dt.float16
    BF16 = mybir.dt.bfloat16
    AF = mybir.ActivationFunctionType
    ALU = mybir.AluOpType

    nc = bacc.Bacc("TRN2", target_bir_lowering=False, debug=False)
    hs_d = nc.dram_tensor("hs", [128, NQ * DTC * QC], F16, kind="ExternalInput").ap()
    wq_d = nc.dram_tensor("wq", [128, DTC * QH * 128], F16, kind="ExternalInput").ap()
    wk_d = nc.dram_tensor("wk", [128, DTC * 128], F16, kind="ExternalInput").ap()
    wv_d = nc.dram_tensor("wv", [128, DTC * 128], F16, kind="ExternalInput").ap()
    wo_d = nc.dram_tensor("wo", [128, QH * D], BF16, kind="ExternalInput").ap()
    cos_d = nc.dram_tensor("cos", [128, S], F16, kind="ExternalInput").ap()
    sin_d = nc.dram_tensor("sin", [128, S], F16, kind="ExternalInput").ap()
    trit_d = nc.dram_tensor("trit", [128, 128], BF16, kind="ExternalInput").ap()
    idb_d = nc.dram_tensor("idb", [128, 128], BF16, kind="ExternalInput").ap()
    out_d = nc.dram_tensor("out", [128, NT * D], BF16, kind="ExternalOutput").ap()

    hs4 = hs_d.rearrange("p (q t s) -> p q t s", q=NQ, t=DTC)
    wq3 = wq_d.rearrange("p (t m) -> p t m", t=DTC)
    wk3 = wk_d.rearrange("p (t m) -> p t m", t=DTC)
    wv3 = wv_d.rearrange("p (t m) -> p t m", t=DTC)
    out3 = out_d.rearrange("p (t d) -> p t d", t=NT)

    with tile.TileContext(nc) as tc:
        with tc.tile_pool(name="consts", bufs=1) as consts, \
             tc.tile_pool(name="persist", bufs=1) as persist:
            cos_sb = consts.tile([128, S], F16)
            sin_sb = consts.tile([128, S], F16)
            trit_sb = consts.tile([128, 128], BF16)
            idb_sb = consts.tile([128, 128], BF16)
            ones_sb = consts.tile([128, 1], BF16)
            one1_sb = consts.tile([1, 1], F32)
            nc.gpsimd.memset(one1_sb, 1.0)
            wq_sb = consts.tile([128, DTC, QH * 128], F16)
            wk_sb = consts.tile([128, DTC, 128], F16)
            wv_sb = consts.tile([128, DTC, 128], F16)
            wo_sb = consts.tile([128, QH, D], BF16)
            nc.gpsimd.memset(ones_sb, 1.0)

            qrot = [persist.tile([128, S], BF16, tag=f"qrot{h}", name=f"qrot{h}")
                    for h in range(QH)]
            krot = persist.tile([128, S], BF16, tag="krot")
            vbf = persist.tile([128, S], BF16, tag="vbf")
            vnat = persist.tile([128, NT * 128], BF16, tag="vnat")
            afin = [persist.tile([128, S], BF16, tag=f"afin{h}", name=f"afin{h}")
                    for h in range(QH)]
            linv_sb = persist.tile([128, NQ, QH, 4], F32, tag="linv")
            lrow_sb = persist.tile([1, QH, NQ, QC], F32, tag="lrow")

            from concourse.tile import add_dep_helper
            vt_dma = [None] * NQ
            vbf_wr = [None] * NQ

            def emit_vnat_xbar(sq):
                # v quarter -> natural [s, hd] blocks via DMA XBAR transpose,
                # issued from the scalar hwdge queue one quarter late so its
                # wait never blocks rope copies or the hs loads on SP.  Its
                # APs are not dep-tracked: order manually.
                cols = slice(sq * QC, (sq + 1) * QC)
                vt_dma[sq] = nc.scalar.dma_start(
                    out=vnat[:, cols].rearrange("p (t f) -> p t f", t=4),
                    in_=vbf[:, cols], transpose=True)
                add_dep_helper(vt_dma[sq].ins, vbf_wr[sq].ins,
                               reason="vnat xbar after vbf quarter")

            # ---------------- QKV projections (+RoPE), 512-col quarters ------
            with tc.tile_pool(name="hsp", bufs=3) as hsp, \
                 tc.tile_pool(name="ropet", bufs=2) as ropet, \
                 tc.tile_pool(name="qkvps", bufs=2, space="PSUM") as qkvps:
                for sq in range(NQ):
                    if sq > 0:
                        emit_vnat_xbar(sq - 1)
                    cols = slice(sq * QC, (sq + 1) * QC)
                    pq = [qkvps.tile([128, QC], F32, tag=f"pq{m}", name=f"pq{m}")
                          for m in range(QH)]
                    pk = qkvps.tile([128, QC], F32, tag="pk")
                    pv = qkvps.tile([128, QC], F32, tag="pv")
                    for g4 in range(4):
                        hst = hsp.tile([128, 4, QC], F16, tag="hst")
                        nc.sync.dma_start(out=hst,
                                          in_=hs4[:, sq, g4 * 4:(g4 + 1) * 4, :])
                        if sq == 0:    # critical-path DMAs first, consts behind
                            ts = slice(g4 * 4, (g4 + 1) * 4)
                            nc.sync.dma_start(out=wq_sb[:, ts, :], in_=wq3[:, ts, :])
                            nc.sync.dma_start(out=wk_sb[:, ts, :], in_=wk3[:, ts, :])
                            nc.sync.dma_start(out=wv_sb[:, ts, :], in_=wv3[:, ts, :])
                            if g4 == 3:   # needed by this quarter's rope below
                                nc.sync.dma_start(out=cos_sb, in_=cos_d)
                                nc.sync.dma_start(out=sin_sb, in_=sin_d)
                        elif sq == 2 and g4 == 0:
                            nc.sync.dma_start(out=trit_sb, in_=trit_d)
                            nc.sync.dma_start(out=idb_sb, in_=idb_d)
                            nc.sync.dma_start(
                                out=wo_sb, in_=wo_d.rearrange("p (h m) -> p h m", h=QH))
                        for t4 in range(4):
                            dt = g4 * 4 + t4
                            st = dt == 0
                            sp = dt == DTC - 1
                            rhs = hst[:, t4, :]
                            for m in range(QH):
                                nc.tensor.matmul(pq[m], wq_sb[:, dt, m * 128:(m + 1) * 128],
                                                 rhs, start=st, stop=sp)
                            nc.tensor.matmul(pk, wk_sb[:, dt, :], rhs, start=st, stop=sp)
                            nc.tensor.matmul(pv, wv_sb[:, dt, :], rhs, start=st, stop=sp)
                    for m in range(QH):
                        _rope(nc, ropet, qrot[m], cols, pq[m], cos_sb, sin_sb, F32, ALU)
                    _rope(nc, ropet, krot, cols, pk, cos_sb, sin_sb, F32, ALU)
                    vbf_wr[sq] = nc.vector.tensor_copy(out=vbf[:, cols], in_=pv)
                emit_vnat_xbar(NQ - 1)

            # ---------------- attention (S^T form) + o_proj, interleaved -----
            # Per chunk the epilogue (l finalize, aout normalize, o_proj) is
            # emitted as small closures smeared between the NEXT chunk's
            # pipeline slots, so no engine stream head-of-line blocks.

            with tc.tile_pool(name="sps", bufs=3, space="PSUM") as sps, \
                 tc.tile_pool(name="pvps", bufs=1, space="PSUM") as pvps, \
                 tc.tile_pool(name="lps", bufs=1, space="PSUM") as lps, \
                 tc.tile_pool(name="ops", bufs=2, space="PSUM") as ops, \
                 tc.tile_pool(name="etp", bufs=5) as etp, \
                 tc.tile_pool(name="smallp", bufs=2) as smallp, \
                 tc.tile_pool(name="osb", bufs=2) as osb:

                def epilogue_closures(c, pv_ps, l_ps):
                    """Per-head emission units for chunk c's epilogue + o_proj.
                    Returns (h0 closures, h1 closures, o_proj closures)."""
                    qcols = slice(c * QC, (c + 1) * QC)
                    st = {}

                    def l_finalize(h):
                        # l row -> [128, 4] columns via K=1 PE transposes
                        # (lhsT = the [1, 128] row, identity = [1, 1]) -> 1/l.
                        # All on-chip; no DMA in the chain.
                        nc.vector.tensor_copy(out=lrow_sb[:, h, c, :],
                                              in_=l_ps[64 * h:64 * h + 1, :])
                        lt = sps.tile([128, QC], F32, tag="s", name="lt")
                        for i in range(4):
                            nc.tensor.transpose(
                                lt[:, i:i + 1],
                                lrow_sb[:, h, c, i * 128:(i + 1) * 128], one1_sb)
                        nc.vector.reciprocal(out=linv_sb[:, c, h, :],
                                             in_=lt[:, 0:4])

                    def pv_copy(h):
                        pv_sb = smallp.tile([128, QC], BF16, tag="pvsb",
                                            name="pv_sb")
                        st[f"pvsb{h}"] = pv_sb
                        nc.vector.tensor_copy(out=pv_sb, in_=pv_ps[h])

                    def a_transpose(h):
                        aT = ops.tile([128, 512], BF16, tag="po", name="aT")
                        st[f"aT{h}"] = aT
                        for i in range(4):
                            nc.tensor.transpose(aT[:, i * 128:(i + 1) * 128],
                                                st[f"pvsb{h}"][:, i * 128:(i + 1) * 128],
                                                idb_sb)

                    def a_scale(h):
                        aN = smallp.tile([128, QC], BF16, tag="ansb", name="aN")
                        st[f"aN{h}"] = aN
                        for i in range(4):
                            nc.scalar.activation(
                                out=aN[:, i * 128:(i + 1) * 128],
                                in_=st[f"aT{h}"][:, i * 128:(i + 1) * 128],
                                func=AF.Copy, scale=linv_sb[:, c, h, i:i + 1])

                    def a_back(h):
                        af = ops.tile([128, 512], BF16, tag="po", name="af")
                        for i in range(4):
                            nc.tensor.transpose(af[:, i * 128:(i + 1) * 128],
                                                st[f"aN{h}"][:, i * 128:(i + 1) * 128],
                                                idb_sb)
                        nc.vector.tensor_copy(out=afin[h][:, qcols], in_=af)

                    def o_proj(i):
                        t = 4 * c + i
                        o_sb = osb.tile([128, D], BF16, tag="osb", name="o_sb")
                        for n in range(D // 512):
                            po = ops.tile([128, 512], F32, tag="po", name="po")
                            for h in range(QH):
                                nc.tensor.matmul(po, afin[h][:, t * 128:(t + 1) * 128],
                                                 wo_sb[:, h, n * 512:(n + 1) * 512],
                                                 start=(h == 0), stop=(h == QH - 1))
                            dst = o_sb[:, n * 512:(n + 1) * 512]
                            if n == 0:
                                nc.scalar.copy(out=dst, in_=po)
                            else:
                                nc.vector.tensor_copy(out=dst, in_=po)
                        nc.sync.dma_start(out=out3[:, t, :], in_=o_sb)

                    head_cl = [[lambda h=h: l_finalize(h), lambda h=h: pv_copy(h),
                                lambda h=h: a_transpose(h), lambda h=h: a_scale(h),
                                lambda h=h: a_back(h)] for h in range(QH)]
                    return head_cl + [[lambda i=i: o_proj(i) for i in range(4)]]

                pending = []
                first_mm_of_chunk = {}
                for c in range(NQ):
                    jmax = 4 * c + 3
                    slots = [(h, j) for h in range(QH) for j in range(jmax + 1)]
                    pv_ps = [pvps.tile([128, QC], F32, tag=f"pv{h}", name=f"pv{h}")
                             for h in range(QH)]
                    l_ps = lps.tile([65, QC], F32, tag="lrow")  # rows at p=0,64
                    closures = epilogue_closures(c, pv_ps, l_ps)
                    pend = []

                    def emit_pvl(h, j, et, qlo, pv_ps=pv_ps, l_ps=l_ps, jmax=jmax,
                                 closures=closures):
                        nc.tensor.matmul(pv_ps[h][:, qlo:QC],
                                         vnat[:, j * 128:(j + 1) * 128],
                                         et[:, qlo:QC],
                                         start=(j == 0), stop=(j == jmax))
                        nc.tensor.matmul(l_ps[64 * h:64 * h + 1, qlo:QC],
                                         ones_sb, et[:, qlo:QC],
                                         start=(j == 0), stop=(j == jmax))
                        if j == jmax:   # head done: its epilogue may smear in
                            pending.extend(closures[h])

                    for (h, j) in slots:
                        qlo = max(0, j * 128 - c * QC)
                        s_ps = sps.tile([128, QC], F32, tag="s")
                        diag = j >= 4 * c
                        mm = nc.tensor.matmul(s_ps[:, qlo:QC],
                                              krot[:, j * 128:(j + 1) * 128],
                                              qrot[h][:, c * QC + qlo:(c + 1) * QC],
                                              start=True, stop=not diag)
                        if c not in first_mm_of_chunk:
                            first_mm_of_chunk[c] = mm
                            add_dep_helper(mm.ins, vt_dma[c].ins,
                                           reason="attention PE after vnat xbar")
                        if diag:   # causal mask: accumulate trit via PE
                            nc.tensor.matmul(s_ps[:, qlo:qlo + 128],
                                             idb_sb, trit_sb,
                                             start=False, stop=True)
                        et = etp.tile([128, QC], BF16, tag="et")
                        nc.scalar.activation(out=et[:, qlo:QC], in_=s_ps[:, qlo:QC],
                                             func=AF.Exp, scale=SCALE)
                        pend.append((h, j, et, qlo))
                        if len(pend) > LAG:
                            emit_pvl(*pend.pop(0))
                        if pending:
                            pending.pop(0)()
                    for args in pend:
                        emit_pvl(*args)
                    pending.extend(closures[QH])
                while pending:
                    pending.pop(0)()

    nc.compile()
    return nc


def _pm(x):
    """[n*128, M] row-major -> partition-major [128, n*M]."""
    n = x.shape[0] // 128
    return np.ascontiguousarray(
        x.reshape(n, 128, x.shape[1]).transpose(1, 0, 2).reshape(128, -1))


def prep_in_maps(hidden_states, position_ids, Wq, Wk, Wv, Wo):
    import ml_dtypes
    hs = np.asarray(hidden_states, dtype=np.float32).reshape(S, D)
    hsT_pm = _pm(np.ascontiguousarray(hs.T))                       # [128, DTC*S]
    # [128, dt, S] -> quarter-major [128, sq, dt, 512] fp16
    hs_q = (hsT_pm.reshape(128, DTC, NQ, QC).transpose(0, 2, 1, 3)
            .reshape(128, -1).astype(np.float16))

    pos = np.asarray(position_ids).reshape(S).astype(np.float32)
    inv = (ROPE_BASE ** (-np.arange(0, HD, 2, dtype=np.float32) / HD))  # [64]
    ang = np.concatenate([pos[None, :] * inv[:, None]] * 2, axis=0)     # [128, S]
    cos_t = np.cos(ang).astype(np.float16)
    sin_t = np.sin(ang)
    sin_signed = np.concatenate([-sin_t[:64], sin_t[64:]], axis=0).astype(np.float16)

    k_idx = np.arange(128)[:, None]   # partition = k
    q_idx = np.arange(128)[None, :]   # col = q
    trit = np.where(q_idx >= k_idx, 0.0, NEG).astype(ml_dtypes.bfloat16)
    idb = np.eye(128, dtype=ml_dtypes.bfloat16)

    Wq = np.asarray(Wq, np.float32)
    Wk = np.asarray(Wk, np.float32)
    Wv = np.asarray(Wv, np.float32)
    Wo = np.asarray(Wo, np.float32)

    in_maps = []
    for c in range(NCORES):
        g = (c * QH) // (H // KV)          # kv head owned by this core
        wq_c = Wq[c * QH * 128:(c + 1) * QH * 128]      # [256, D]
        wk_c = Wk[g * 128:(g + 1) * 128]                # [128, D]
        wv_c = Wv[g * 128:(g + 1) * 128]                # [128, D]
        wo_c = Wo[:, c * QH * 128:(c + 1) * QH * 128]   # [D, 256]
        in_maps.append({
            "hs": hs_q,
            "wq": _pm(np.ascontiguousarray(wq_c.T)).astype(np.float16),
            "wk": _pm(np.ascontiguousarray(wk_c.T)).astype(np.float16),
            "wv": _pm(np.ascontiguousarray(wv_c.T)).astype(np.float16),
            "wo": _pm(np.ascontiguousarray(wo_c.T)).astype(ml_dtypes.bfloat16),
            "cos": cos_t,
            "sin": sin_signed,
            "trit": trit,
            "idb": idb,
        })
    return in_maps


def combine_outputs(results):
    total = np.zeros((S, D), np.float32)
    for r in results:
        o = np.asarray(r["out"]).astype(np.float32)
        total += o.reshape(128, NT, D).transpose(1, 0, 2).reshape(S, D)
    return total[None]


def kernel(hidden_states, attention_mask, position_ids, Wq, Wk, Wv, Wo):
    from concourse import bass_utils
    if "nc" not in _CACHE:
        _CACHE["nc"] = build_nc()
    nc = _CACHE["nc"]
    in_maps = prep_in_maps(hidden_states, position_ids, Wq, Wk, Wv, Wo)
    res = bass_utils.run_bass_kernel_spmd(nc, in_maps, core_ids=list(range(NCORES)))
    return combine_outputs(res.results)
